# revision 59
# baseline (speedup 1.0000x reference)
"""Trainium2 Bass kernel for nn_NUFFTLayerMultiChannelInitMixed.

Math: the reference's spread->FFT->filter->IFFT->energy pipeline collapses to
an analytic-spectrum bilinear form. The Gaussian spread is deconvolved exactly
by the deconv^2 filter, so with ghat_n(k) ~ e^{-i k x_n} (alias images carry
weight e^{-tau(M-k)^2} ~ 3e-5 -- negligible vs the 2e-2 gate):

  e_i[n] = sum_k G_i(k) [cos(k x_n) C(k) + sin(k x_n) S(k)] + off_i
  C(k) = sum_n cos(k x_n),  S(k) = sum_n sin(k x_n)
  G_i = pref * w * deconv2 * mult_i * p^2  (~1/k^2 decay; K=32 keeps ~2e-4)

Layout: with K=32, BOTH batches pack into one [128, 1024] trig matrix --
row r: batch r//64, kind (r%64)//32 (cos/sin), k = r%32. One K=5 fp16 phase
matmul per 512 cols builds k*t (2-way fp16 split of t, low part pre-scaled
by 2^11 to dodge denormals; exact to ~2^-23 in fp32 PSUM) with the +1/4-turn
cos bias riding in the contraction. A fused custom DVE op range-reduces
(r = u - round(u) via the +MAGIC trick), one Sin activation per half (fp16
out, accum_out = row sums for free), a fused custom DVE op forms
UC = G*(C+S sums), then 16 matmuls (stationary = 64-row trig chunk, rhs =
UC [64, 2]) yield energies directly in [n-part, channel] layout, staged to
DRAM in fp16 (metric has 2e-2 slack). The constant per-channel offset rides
the two identically-1 trig rows (cos k=0, and sin k=0 via bias).
Sharding: batch-parallel, 2 of 16 batches per core, no collectives.
"""

import numpy as np

try:
    import concourse.bass as bass
except ImportError:
    import sys
    sys.path.insert(0, "/opt/trn_rl_repo")
    import concourse.bass as bass

import concourse.bacc as bacc
import concourse.mybir as mybir
from concourse import tile
from concourse.bass_utils import run_bass_kernel_spmd
from concourse.dve_spec import Spec, Src0, C0, C1, lower as _dve_lower
import concourse.dve_ops as _DO


def _register_op(name, spec):
    """Register (once per process) a custom DVE op with computed uops_sha."""
    for op in _DO.OPS:
        if op.name == name:
            return op
    op = _DO.DveOp(name, spec, subdim=False, uops_sha={})
    _DO.OPS.append(op)
    _DO.CUSTOM_DVE_SPECS[name] = spec
    _DO._SUB_OPCODE_FOR_NAME[name] = _DO._CUSTOM_DVE_ROW_BASE + len(_DO.OPS) - 1
    for ver in ("v3", "v4"):
        uops = _dve_lower(spec, ver=ver)
        r = _DO.DveOpSpec(name=name, opcode=_DO.get_dve_sub_opcode(name),
                          uops=uops, rd1_en=False)
        op.uops_sha[ver] = r.sha(ver)
    return op


# Fused range reduction: out = in0 - ((in0 + s0) - s0); with s0 = 1.5*2^23
# the inner add rounds to the nearest integer in fp32, so out =
# in0 - round(in0) in [-0.5, 0.5]. Replaces a 2-instruction round+subtract
# pass (validated bit-exact on HW).
_ub = Src0 + C1
_FRAC = _register_op(
    "FRAC_CENTER_BIAS_ANT",
    Spec(body=_ub - ((_ub + C0) - C0),
         reference=lambda in0, in1, s0, s1, imm2: (in0 + s1)
         - ((in0 + s1 + s0) - s0)))

# Fused UC: out = in0*s0 + in0*s1 with per-partition scalars; replaces a
# mult + a scalar_tensor_tensor pair.
_MULSUM = _register_op(
    "MUL_SCALAR_SUM_ANT",
    Spec(body=Src0 * C0 + Src0 * C1,
         reference=lambda in0, in1, s0, s1, imm2: in0 * s0 + in0 * s1))

F32 = mybir.dt.float32
BF16 = mybir.dt.bfloat16
AF = mybir.ActivationFunctionType
ALU = mybir.AluOpType

M = 2001
L = 2.0 * np.pi
TAU = 12.0 * (L / (2.0 * np.pi * M)) ** 2
K = 32                   # spectral truncation (1/k^2 filter decay)
N = 1024
B_FULL = 16
NCORES = 8
BPC = B_FULL // NCORES   # batches per core, packed into row halves
MAGIC = 12582912.0       # 1.5 * 2^23: (u + MAGIC) - MAGIC = round-to-nearest(u)
PI = float(np.pi)

_RB = np.arange(128) % 64            # within-batch row index
_KROW = _RB % K                      # k value per row
_BIAS = np.where(_RB <= K, 0.25, 0.0)  # cos rows + the sin k=0 offset row


def _bf16(a):
    a32 = np.asarray(a, dtype=np.float32)
    u32 = a32.view(np.uint32).astype(np.uint64)
    return (((u32 + 0x7FFF + ((u32 >> 16) & 1)) & 0xFFFF0000)
            .astype(np.uint32)).view(np.float32)


def _host_constants(shift0, shift1, amp0, amp1):
    """fp64 host-side k-space weights -> cst2 [128, 2]."""
    k = np.arange(K, dtype=np.float64)
    tau = float(TAU)
    p2 = np.exp(-2.0 * tau * k * k)
    deconv2 = (np.pi / tau) * np.exp(2.0 * tau * k * k)
    mult1 = float(amp0) * (4.0 * np.pi) / (k * k + (1.0 * float(shift0)) ** 2)
    mult2 = float(amp1) * (4.0 * np.pi) / (k * k + (0.5 * float(shift1)) ** 2)
    w = np.full(K, 2.0)
    w[0] = 1.0
    Cc = (M / L) * np.sqrt(4.0 * np.pi * tau)
    scale = 1.0 / ((2.0 * np.pi * M / L) * (2.0 * np.pi))
    pref = scale * Cc * Cc / M
    G1 = pref * w * deconv2 * mult1 * p2
    G2 = pref * w * deconv2 * mult2 * p2

    cst2 = np.zeros((128, 3), dtype=np.float64)
    cst2[:, 0] = G1[_KROW]
    cst2[:, 1] = G2[_KROW]
    cst2[:, 2] = _BIAS

    # Constant offset off_i = G_i[0]*N - sum(G_i) rides the two rows that are
    # identically 1: cos k=0 (rb=0, bf16-representable part) and sin k=0
    # (rb=K, made 1 by its +0.25 bias; carries the residual) -- no separate
    # offset-add instruction and no bf16 precision loss.
    off1 = float(G1[0] * N - G1.sum())
    off2 = float(G2[0] * N - G2.sum())
    for i, off in enumerate((off1, off2)):
        hi = float(_bf16(np.float32(off)))
        cst2[_RB == 0, i] = hi / N
        cst2[_RB == K, i] = (off - hi) / N
    return cst2.astype(np.float32)


def _pack_t(t_rows):
    """[BPC, N] fp32 t values -> [4, 128+N] fp16: the phase stationary
    [4, 128] at cols 0:128 (first in the DMA), then per-batch 2-way fp16
    split rows of t. The low split is pre-scaled by 2^11 (and the stationary
    k by 2^-11, both exact) to keep it out of fp16 denormal range; two
    11-bit splits reproduce t to ~2^-23. The cos bias is applied inside the
    fused FRAC op, so no ones row is needed."""
    ext = np.zeros((4, 128 + N), dtype=np.float32)
    for b in range(BPC):
        t = t_rows[b].astype(np.float64)
        th = t.astype(np.float16)
        tl = ((t - th.astype(np.float64)) * 2048.0).astype(np.float16)
        ext[2 * b + 0, 128:] = th.astype(np.float32)
        ext[2 * b + 1, 128:] = tl.astype(np.float32)
    kvb = np.zeros((4, 128), dtype=np.float64)
    for b in range(BPC):
        rows = (np.arange(128) // 64) == b
        kvb[2 * b + 0, rows] = _KROW[rows]
        kvb[2 * b + 1, rows] = _KROW[rows] / 2048.0
    ext[:, :128] = kvb
    return ext.astype(np.float16)


def _build_program(debug=False):
    nc = bacc.Bacc(None, target_bir_lowering=False, debug=debug)
    t_in = nc.declare_dram_parameter("t", [4, 128 + N], mybir.dt.float16,
                                     isOutput=False)
    cst_in = nc.declare_dram_parameter("cst2", [128, 3], F32, isOutput=False)
    out_t = nc.declare_dram_parameter("out", [128, 16 * BPC], mybir.dt.float16,
                                      isOutput=True)

    with tile.TileContext(nc) as tc:
        import contextlib
        with contextlib.ExitStack() as ctx:
            pc = ctx.enter_context(tc.tile_pool(name="const", bufs=1))
            wp = ctx.enter_context(tc.tile_pool(name="work", bufs=2))
            sp = ctx.enter_context(tc.tile_pool(name="small", bufs=1))
            ps_u = ctx.enter_context(tc.tile_pool(name="psu", bufs=2, space="PSUM"))
            ps_T = ctx.enter_context(tc.tile_pool(name="psT", bufs=1, space="PSUM"))

            # Dummy Sin on scratch: makes the FIRST ScalarE op a Sin so the
            # compiler resident-set pick contains sin (its sets also contain
            # identity), avoiding a 1.3us mid-pipeline ACT_TABLE_LOAD swap.
            dummy = sp.tile([1, 2], F32, tag="dummy")
            nc.vector.memset(dummy[:], 0.0)
            dummy2 = sp.tile([1, 2], F32, tag="dummy2")
            nc.scalar.activation(dummy2[:], dummy[:], AF.Sin, scale=1.0)

            cst2 = pc.tile([128, 3], F32, tag="cst2")
            nc.sync.dma_start(cst2[:], cst_in[:])
            t_ext = pc.tile([4, 128 + N], mybir.dt.float16, tag="t")
            nc.sync.dma_start(t_ext[:, 0:640], t_in[:, 0:640])
            nc.sync.dma_start(t_ext[:, 640:], t_in[:, 640:])
            kvb = t_ext[:, 0:128]

            CS = sp.tile([128, N], mybir.dt.float16, tag="CS")
            csum = sp.tile([128, 2], F32, tag="csum")

            u0 = ps_u.tile([128, 512], F32, tag="u")
            nc.tensor.matmul(u0[:], kvb, t_ext[:, 128:640], start=True, stop=True)
            u1 = ps_u.tile([128, 512], F32, tag="u")
            nc.tensor.matmul(u1[:], kvb, t_ext[:, 640:1152], start=True, stop=True)

            # Fused range reduction: one custom DVE op per half computes
            # r = u - round(u) in [-0.5, 0.5] directly from the PSUM phase.
            r0 = wp.tile([128, 512], F32, tag="r0")
            nc.vector._custom_dve(_FRAC, out=r0[:], in0=u0[:], s0=MAGIC,
                                  s1=cst2[:, 2:3])
            nc.scalar.activation(CS[:, 0:512], r0[:], AF.Sin, scale=2.0 * PI,
                                 accum_out=csum[:, 0:1])
            r1 = wp.tile([128, 512], F32, tag="r1")
            nc.vector._custom_dve(_FRAC, out=r1[:], in0=u1[:], s0=MAGIC,
                                  s1=cst2[:, 2:3])
            nc.scalar.activation(CS[:, 512:1024], r1[:], AF.Sin,
                                 scale=2.0 * PI, accum_out=csum[:, 1:2])

            # T-side in two accumulation passes with partial UCs: by
            # bilinearity T = sum_k CS*(G*c0) + sum_k CS*(G*c1), so the first
            # 16 matmuls run with UC0 right after half 0's accumulator read,
            # entirely inside otherwise-idle PE time; only the UC1 pass waits
            # for half 1.
            # UC = cst2 * (csum0 + csum1) in one fused custom DVE op
            UC = sp.tile([128, 2], mybir.dt.float16, tag="UC")
            nc.vector._custom_dve(_MULSUM, out=UC[:], in0=cst2[:],
                                  s0=csum[:, 0:1], s1=csum[:, 1:2])

            pT = ps_T.tile([128, 16 * BPC], F32, tag="pT")
            # e staged in fp16: halves the out-DMA bytes (the issue cost is
            # byte-proportional and on the critical tail); fp16's 10-bit
            # mantissa adds only ~5e-4 relative error vs the 2e-2 gate
            # (energies max ~16k, well under fp16 max 65504).
            e = sp.tile([128, 16 * BPC], mybir.dt.float16, tag="e")
            for b in range(BPC):
                for j in range(8):
                    lh = CS[64 * b:64 * (b + 1), 128 * j:128 * (j + 1)]
                    nc.tensor.matmul(pT[:, 16 * b + 2 * j: 16 * b + 2 * j + 2],
                                     lh, UC[64 * b:64 * (b + 1), :],
                                     start=True, stop=True)
                nc.vector.tensor_copy(e[:, 16 * b:16 * (b + 1)],
                                      pT[:, 16 * b:16 * (b + 1)])
                nc.sync.dma_start(out_t[:, 16 * b:16 * (b + 1)],
                                  e[:, 16 * b:16 * (b + 1)])
    return nc


def kernel(x, shift0, shift1, amp0, amp1):
    x = np.asarray(x, dtype=np.float32)
    cst2 = _host_constants(
        np.asarray(shift0).reshape(-1)[0], np.asarray(shift1).reshape(-1)[0],
        np.asarray(amp0).reshape(-1)[0], np.asarray(amp1).reshape(-1)[0])
    nc = _build_program()
    nc.finalize()

    t_full = (x.astype(np.float64) / (2.0 * np.pi)).astype(np.float32)
    in_maps = []
    for c in range(NCORES):
        t_ext = _pack_t(t_full[BPC * c: BPC * (c + 1)])
        in_maps.append({"t": t_ext, "cst2": cst2})
    res = run_bass_kernel_spmd(nc, in_maps, list(range(NCORES)))
    outs = []
    for c in range(NCORES):
        arr = np.asarray(res.results[c]["out"], dtype=np.float32)
        arr = arr.reshape(128, BPC, 8, 2)                # (p, b, j, i)
        outs.append(arr.transpose(1, 2, 0, 3).reshape(BPC, N, 2))
    return np.concatenate(outs, axis=0).astype(np.float32)


# revision 60
# speedup vs baseline: 1.0144x; 1.0144x over previous
"""Trainium2 Bass kernel for nn_NUFFTLayerMultiChannelInitMixed.

Math: the reference's spread->FFT->filter->IFFT->energy pipeline collapses to
an analytic-spectrum bilinear form. The Gaussian spread is deconvolved exactly
by the deconv^2 filter, so with ghat_n(k) ~ e^{-i k x_n} (alias images carry
weight e^{-tau(M-k)^2} ~ 3e-5 -- negligible vs the 2e-2 gate):

  e_i[n] = sum_k G_i(k) [cos(k x_n) C(k) + sin(k x_n) S(k)] + off_i
  C(k) = sum_n cos(k x_n),  S(k) = sum_n sin(k x_n)
  G_i = pref * w * deconv2 * mult_i * p^2  (~1/k^2 decay; K=32 keeps ~2e-4)

Layout: with K=32, BOTH batches pack into one [128, 1024] trig matrix --
row r: batch r//64, kind (r%64)//32 (cos/sin), k = r%32. One K=5 fp16 phase
matmul per 512 cols builds k*t (2-way fp16 split of t, low part pre-scaled
by 2^11 to dodge denormals; exact to ~2^-23 in fp32 PSUM) with the +1/4-turn
cos bias riding in the contraction. A fused custom DVE op range-reduces
(r = u - round(u) via the +MAGIC trick), one Sin activation per half (fp16
out, accum_out = row sums for free), a fused custom DVE op forms
UC = G*(C+S sums), then 16 matmuls (stationary = 64-row trig chunk, rhs =
UC [64, 2]) yield energies directly in [n-part, channel] layout, staged to
DRAM in fp16 (metric has 2e-2 slack). The constant per-channel offset rides
the two identically-1 trig rows (cos k=0, and sin k=0 via bias).
Sharding: batch-parallel, 2 of 16 batches per core, no collectives.
"""

import numpy as np

try:
    import concourse.bass as bass
except ImportError:
    import sys
    sys.path.insert(0, "/opt/trn_rl_repo")
    import concourse.bass as bass

import concourse.bacc as bacc
import concourse.mybir as mybir
from concourse import tile
from concourse.bass_utils import run_bass_kernel_spmd
from concourse.dve_spec import Spec, Src0, C0, C1, lower as _dve_lower
import concourse.dve_ops as _DO


def _register_op(name, spec):
    """Register (once per process) a custom DVE op with computed uops_sha."""
    for op in _DO.OPS:
        if op.name == name:
            return op
    op = _DO.DveOp(name, spec, subdim=False, uops_sha={})
    _DO.OPS.append(op)
    _DO.CUSTOM_DVE_SPECS[name] = spec
    _DO._SUB_OPCODE_FOR_NAME[name] = _DO._CUSTOM_DVE_ROW_BASE + len(_DO.OPS) - 1
    for ver in ("v3", "v4"):
        uops = _dve_lower(spec, ver=ver)
        r = _DO.DveOpSpec(name=name, opcode=_DO.get_dve_sub_opcode(name),
                          uops=uops, rd1_en=False)
        op.uops_sha[ver] = r.sha(ver)
    return op


# Fused range reduction: out = in0 - ((in0 + s0) - s0); with s0 = 1.5*2^23
# the inner add rounds to the nearest integer in fp32, so out =
# in0 - round(in0) in [-0.5, 0.5]. Replaces a 2-instruction round+subtract
# pass (validated bit-exact on HW).
_ub = Src0 + C1
_FRAC = _register_op(
    "FRAC_CENTER_BIAS_ANT",
    Spec(body=_ub - ((_ub + C0) - C0),
         reference=lambda in0, in1, s0, s1, imm2: (in0 + s1)
         - ((in0 + s1 + s0) - s0)))

# Fused UC: out = in0*s0 + in0*s1 with per-partition scalars; replaces a
# mult + a scalar_tensor_tensor pair.
_MULSUM = _register_op(
    "MUL_SCALAR_SUM_ANT",
    Spec(body=Src0 * C0 + Src0 * C1,
         reference=lambda in0, in1, s0, s1, imm2: in0 * s0 + in0 * s1))

F32 = mybir.dt.float32
BF16 = mybir.dt.bfloat16
AF = mybir.ActivationFunctionType
ALU = mybir.AluOpType

M = 2001
L = 2.0 * np.pi
TAU = 12.0 * (L / (2.0 * np.pi * M)) ** 2
K = 32                   # spectral truncation (1/k^2 filter decay)
N = 1024
B_FULL = 16
NCORES = 8
BPC = B_FULL // NCORES   # batches per core, packed into row halves
MAGIC = 12582912.0       # 1.5 * 2^23: (u + MAGIC) - MAGIC = round-to-nearest(u)
PI = float(np.pi)

_RB = np.arange(128) % 64            # within-batch row index
_KROW = _RB % K                      # k value per row
_BIAS = np.where(_RB <= K, 0.25, 0.0)  # cos rows + the sin k=0 offset row


def _bf16(a):
    a32 = np.asarray(a, dtype=np.float32)
    u32 = a32.view(np.uint32).astype(np.uint64)
    return (((u32 + 0x7FFF + ((u32 >> 16) & 1)) & 0xFFFF0000)
            .astype(np.uint32)).view(np.float32)


def _host_constants(shift0, shift1, amp0, amp1):
    """fp64 host-side k-space weights -> cst2 [128, 2]."""
    k = np.arange(K, dtype=np.float64)
    tau = float(TAU)
    p2 = np.exp(-2.0 * tau * k * k)
    deconv2 = (np.pi / tau) * np.exp(2.0 * tau * k * k)
    mult1 = float(amp0) * (4.0 * np.pi) / (k * k + (1.0 * float(shift0)) ** 2)
    mult2 = float(amp1) * (4.0 * np.pi) / (k * k + (0.5 * float(shift1)) ** 2)
    w = np.full(K, 2.0)
    w[0] = 1.0
    Cc = (M / L) * np.sqrt(4.0 * np.pi * tau)
    scale = 1.0 / ((2.0 * np.pi * M / L) * (2.0 * np.pi))
    pref = scale * Cc * Cc / M
    G1 = pref * w * deconv2 * mult1 * p2
    G2 = pref * w * deconv2 * mult2 * p2

    cst2 = np.zeros((128, 3), dtype=np.float64)
    cst2[:, 0] = G1[_KROW]
    cst2[:, 1] = G2[_KROW]
    cst2[:, 2] = _BIAS

    # Constant offset off_i = G_i[0]*N - sum(G_i) rides the two rows that are
    # identically 1: cos k=0 (rb=0, bf16-representable part) and sin k=0
    # (rb=K, made 1 by its +0.25 bias; carries the residual) -- no separate
    # offset-add instruction and no bf16 precision loss.
    off1 = float(G1[0] * N - G1.sum())
    off2 = float(G2[0] * N - G2.sum())
    for i, off in enumerate((off1, off2)):
        hi = float(_bf16(np.float32(off)))
        cst2[_RB == 0, i] = hi / N
        cst2[_RB == K, i] = (off - hi) / N
    return cst2.astype(np.float32)


def _pack_t(t_rows):
    """[BPC, N] fp32 t values -> [4, 128+N] fp16: the phase stationary
    [4, 128] at cols 0:128 (first in the DMA), then per-batch 2-way fp16
    split rows of t. The low split is pre-scaled by 2^11 (and the stationary
    k by 2^-11, both exact) to keep it out of fp16 denormal range; two
    11-bit splits reproduce t to ~2^-23. The cos bias is applied inside the
    fused FRAC op, so no ones row is needed."""
    ext = np.zeros((4, 128 + N), dtype=np.float32)
    for b in range(BPC):
        t = t_rows[b].astype(np.float64)
        th = t.astype(np.float16)
        tl = ((t - th.astype(np.float64)) * 2048.0).astype(np.float16)
        ext[2 * b + 0, 128:] = th.astype(np.float32)
        ext[2 * b + 1, 128:] = tl.astype(np.float32)
    kvb = np.zeros((4, 128), dtype=np.float64)
    for b in range(BPC):
        rows = (np.arange(128) // 64) == b
        kvb[2 * b + 0, rows] = _KROW[rows]
        kvb[2 * b + 1, rows] = _KROW[rows] / 2048.0
    ext[:, :128] = kvb
    return ext.astype(np.float16)


def _build_program(debug=False):
    nc = bacc.Bacc(None, target_bir_lowering=False, debug=debug)
    t_in = nc.declare_dram_parameter("t", [4, 128 + N], mybir.dt.float16,
                                     isOutput=False)
    cst_in = nc.declare_dram_parameter("cst2", [128, 3], F32, isOutput=False)
    out_t = nc.declare_dram_parameter("out", [128, 16 * BPC], mybir.dt.float16,
                                      isOutput=True)

    with tile.TileContext(nc) as tc:
        import contextlib
        with contextlib.ExitStack() as ctx:
            pc = ctx.enter_context(tc.tile_pool(name="const", bufs=1))
            wp = ctx.enter_context(tc.tile_pool(name="work", bufs=2))
            sp = ctx.enter_context(tc.tile_pool(name="small", bufs=1))
            ps_u = ctx.enter_context(tc.tile_pool(name="psu", bufs=2, space="PSUM"))
            ps_T = ctx.enter_context(tc.tile_pool(name="psT", bufs=1, space="PSUM"))

            # Dummy Sin on scratch: makes the FIRST ScalarE op a Sin so the
            # compiler resident-set pick contains sin (its sets also contain
            # identity), avoiding a 1.3us mid-pipeline ACT_TABLE_LOAD swap.
            dummy = sp.tile([1, 2], F32, tag="dummy")
            nc.vector.memset(dummy[:], 0.0)
            dummy2 = sp.tile([1, 2], F32, tag="dummy2")
            nc.scalar.activation(dummy2[:], dummy[:], AF.Sin, scale=1.0)

            cst2 = pc.tile([128, 3], F32, tag="cst2")
            nc.sync.dma_start(cst2[:], cst_in[:])
            t_ext = pc.tile([4, 128 + N], mybir.dt.float16, tag="t")
            nc.sync.dma_start(t_ext[:, 0:640], t_in[:, 0:640])
            nc.sync.dma_start(t_ext[:, 640:], t_in[:, 640:])
            kvb = t_ext[:, 0:128]

            CS = sp.tile([128, N], mybir.dt.float16, tag="CS")
            csum = sp.tile([128, 2], F32, tag="csum")

            u0 = ps_u.tile([128, 512], F32, tag="u")
            nc.tensor.matmul(u0[:], kvb, t_ext[:, 128:640], start=True, stop=True)
            u1 = ps_u.tile([128, 512], F32, tag="u")
            nc.tensor.matmul(u1[:], kvb, t_ext[:, 640:1152], start=True, stop=True)

            # Fused range reduction: one custom DVE op per half computes
            # r = u - round(u) in [-0.5, 0.5] directly from the PSUM phase.
            r0 = wp.tile([128, 512], F32, tag="r0")
            nc.vector._custom_dve(_FRAC, out=r0[:], in0=u0[:], s0=MAGIC,
                                  s1=cst2[:, 2:3])
            nc.scalar.activation(CS[:, 0:512], r0[:], AF.Sin, scale=2.0 * PI,
                                 accum_out=csum[:, 0:1])
            r1 = wp.tile([128, 512], F32, tag="r1")
            nc.vector._custom_dve(_FRAC, out=r1[:], in0=u1[:], s0=MAGIC,
                                  s1=cst2[:, 2:3])
            nc.scalar.activation(CS[:, 512:1024], r1[:], AF.Sin,
                                 scale=2.0 * PI, accum_out=csum[:, 1:2])

            # T-side in two accumulation passes with partial UCs: by
            # bilinearity T = sum_k CS*(G*c0) + sum_k CS*(G*c1), so the first
            # 16 matmuls run with UC0 right after half 0's accumulator read,
            # entirely inside otherwise-idle PE time; only the UC1 pass waits
            # for half 1.
            # UC = cst2 * (csum0 + csum1) in one fused custom DVE op
            UC = sp.tile([128, 2], mybir.dt.float16, tag="UC")
            nc.vector._custom_dve(_MULSUM, out=UC[:], in0=cst2[:, 0:2],
                                  s0=csum[:, 0:1], s1=csum[:, 1:2])

            pT = ps_T.tile([128, 16 * BPC], F32, tag="pT")
            # e staged in fp16: halves the out-DMA bytes (the issue cost is
            # byte-proportional and on the critical tail); fp16's 10-bit
            # mantissa adds only ~5e-4 relative error vs the 2e-2 gate
            # (energies max ~16k, well under fp16 max 65504).
            e = sp.tile([128, 16 * BPC], mybir.dt.float16, tag="e")
            for b in range(BPC):
                for j in range(8):
                    lh = CS[64 * b:64 * (b + 1), 128 * j:128 * (j + 1)]
                    nc.tensor.matmul(pT[:, 16 * b + 2 * j: 16 * b + 2 * j + 2],
                                     lh, UC[64 * b:64 * (b + 1), :],
                                     start=True, stop=True)
                nc.vector.tensor_copy(e[:, 16 * b:16 * (b + 1)],
                                      pT[:, 16 * b:16 * (b + 1)])
                nc.sync.dma_start(out_t[:, 16 * b:16 * (b + 1)],
                                  e[:, 16 * b:16 * (b + 1)])
    return nc


def kernel(x, shift0, shift1, amp0, amp1):
    x = np.asarray(x, dtype=np.float32)
    cst2 = _host_constants(
        np.asarray(shift0).reshape(-1)[0], np.asarray(shift1).reshape(-1)[0],
        np.asarray(amp0).reshape(-1)[0], np.asarray(amp1).reshape(-1)[0])
    nc = _build_program()
    nc.finalize()

    t_full = (x.astype(np.float64) / (2.0 * np.pi)).astype(np.float32)
    in_maps = []
    for c in range(NCORES):
        t_ext = _pack_t(t_full[BPC * c: BPC * (c + 1)])
        in_maps.append({"t": t_ext, "cst2": cst2})
    res = run_bass_kernel_spmd(nc, in_maps, list(range(NCORES)))
    outs = []
    for c in range(NCORES):
        arr = np.asarray(res.results[c]["out"], dtype=np.float32)
        arr = arr.reshape(128, BPC, 8, 2)                # (p, b, j, i)
        outs.append(arr.transpose(1, 2, 0, 3).reshape(BPC, N, 2))
    return np.concatenate(outs, axis=0).astype(np.float32)
